# revision 1
# baseline (speedup 1.0000x reference)
"""DeepSeek-V3-style MoE gate (nn_MoEGate) on 8 Trainium2 NeuronCores.

Data-parallel: token dim (16384) sharded 8 ways. Per core (2048 tokens):

  - Hidden states are pre-transposed/tiled on the HOST so every SBUF tile
    lands with the contraction (hidden) dim on partitions — no on-chip
    transposes at all.
  - The fp32 gating GEMM is decomposed into two engine-native passes that
    together reproduce fp32-grade routing accuracy:
      pass 1 (fp16, 1 cyc/row):  fp16(hs) @ fp16(w)        -> PSUM bank A
      pass 2 (fp8 DoubleRow, 0.5 cyc/row, K=256 per matmul):
          [fp8((hs - fp16(hs)) * 2^13), fp8(hs)] @ [fp8(w), fp8((w - fp16(w)) * 2^13)]
        i.e. the two first-order residual terms, concatenated along the
        contraction dim                                     -> PSUM bank B
    DVE merges: logits = A + B * 2^-13.  fp16 operands pass through the
    PE's FP22 datapath losslessly, so the only noise left is ~2^-15-level
    (measured: 9 flipped indices of 131072 vs the fp32 reference).
  - ACT applies sigmoid; the DeepSeek group-limited top-k routing epilogue
    runs on DVE (Max8/MaxIndex8, fused group-mask scalar_tensor_tensor,
    iota-compare gathers with fused row-reduce).
  - hs tile DMAs are issued from the ACT queue right after the sigmoid of
    the tile whose SBUF buffers they recycle, so issue order alone
    guarantees the WAR hazard and the DMAs need no cross-engine waits.

This walrus build only accepts ONE semaphore wait per instruction, so every
instruction keeps at most one foreign-engine dependency; extra dependencies
are absorbed by 1-element "touch" instructions on the consuming engine
ordered before the real consumer.
"""

import numpy as np
from contextlib import ExitStack

import concourse.bass as bass
import concourse.tile as tile
import concourse.mybir as mybir
from concourse.bass_utils import run_bass_kernel_spmd
from concourse.tile import add_dep_helper

# problem constants (hardcoded per contract)
S_FULL = 16384
H = 7168
E = 256
N_CORES = 8
SL = S_FULL // N_CORES        # 2048 tokens per core
P = 128                       # partitions
HT = H // P                   # 56 k-tiles
ST = SL // P                  # 16 token-tiles per core
G, GS, TOP_K = 8, 32, 8
TOPK_GROUP = 4
ROUTED_SCALING = 2.5
EPS = 1e-20
HS_BUFS = 4                   # hs tile buffer depth
WCH = 8                       # w DMA chunk count
WCK = HT // WCH               # k-tiles per w chunk
RES_SCALE = 2.0 ** 13         # residual-term scaling for fp8 range
RES_ISCALE = float(2.0 ** -13)

f32 = mybir.dt.float32
f16 = mybir.dt.float16
f8 = mybir.dt.float8e4
u32 = mybir.dt.uint32
i32 = mybir.dt.int32


def _dep(a, b, sync=True, reason="dep"):
    if a is None or b is None:
        return
    add_dep_helper(a.ins if hasattr(a, "ins") else a,
                   b.ins if hasattr(b, "ins") else b, sync=sync, reason=reason)


def _funnel(nc, insts, junk_tile):
    """Serialize kernel-tail dependencies through real SP store instructions
    (NOP funnels are wait-transparent and would let the drain race the
    output DMAs)."""
    for n, inst in enumerate(x for x in insts if x is not None):
        st = nc.sync.store(junk_tile[0:1, n:n + 1], 0)
        _dep(st, inst, sync=True, reason="tail funnel")


def build_program():
    nc = bass.Bass("TRN2", target_bir_lowering=False, debug=False,
                   num_devices=N_CORES)
    hsT16 = nc.dram_tensor("hsT16", [ST, P, H], f16, kind="ExternalInput").ap()
    hsC8 = nc.dram_tensor("hsC8", [ST, P, 2 * H], f8, kind="ExternalInput").ap()
    whi16 = nc.dram_tensor("whi16", [P, HT * E], f16, kind="ExternalInput").ap()
    w8 = nc.dram_tensor("w8", [P, 2 * HT * E], f8, kind="ExternalInput").ap()
    bias = nc.dram_tensor("bias", [P, E], f32, kind="ExternalInput").ap()
    iota = nc.dram_tensor("iota", [P, E], f32, kind="ExternalInput").ap()
    o_w = nc.dram_tensor("o_w", [SL, TOP_K], f32, kind="ExternalOutput").ap()
    o_i = nc.dram_tensor("o_i", [SL, TOP_K], i32, kind="ExternalOutput").ap()

    LAG = 2
    with tile.TileContext(nc) as tc, ExitStack() as ctx:
        const = ctx.enter_context(tc.tile_pool(name="const", bufs=1))
        hsp = ctx.enter_context(tc.tile_pool(name="hsp", bufs=HS_BUFS))
        sco = ctx.enter_context(tc.tile_pool(name="sco", bufs=2))
        lps = ctx.enter_context(tc.tile_pool(name="lps", bufs=LAG + 2, space="PSUM"))
        cps = ctx.enter_context(tc.tile_pool(name="cps", bufs=2, space="PSUM"))
        dps = ctx.enter_context(tc.tile_pool(name="dps", bufs=1, space="PSUM"))

        whi_sb = const.tile([P, HT, E], f16)
        for c in range(WCH):
            nc.sync.dma_start(
                out=whi_sb[:, c * WCK:(c + 1) * WCK, :],
                in_=whi16[:, c * WCK * E:(c + 1) * WCK * E]
                .rearrange("p (k e) -> p k e", e=E))
        w8_sb = const.tile([P, 2 * HT, E], f8)
        for c in range(WCH):
            nc.sync.dma_start(
                out=w8_sb[:, c * 2 * WCK:(c + 1) * 2 * WCK, :],
                in_=w8[:, c * 2 * WCK * E:(c + 1) * 2 * WCK * E]
                .rearrange("p (k e) -> p k e", e=E))
        bias_sb = const.tile([P, E], f32)
        nc.sync.dma_start(out=bias_sb, in_=bias)
        iota_sb = const.tile([P, E], f32)
        nc.sync.dma_start(out=iota_sb, in_=iota)

        strip_names = []
        gated_dmas = []
        pe_only_dmas = []
        wacc = const.tile([P, ST, TOP_K], f32)
        iacc = const.tile([P, ST, TOP_K], i32)
        junk = const.tile([P, 8], f32)
        junk_act = const.tile([P, ST], f32)
        junk_dve = const.tile([P, 2 * ST], f32)
        junk_spg = const.tile([P, ST], i32)
        junk_sp = const.tile([P, 32], i32)

        # DVE observes the bias/iota DMA lanes once
        tch_b = nc.vector.tensor_copy(junk[0:1, 0:1], bias_sb[0:1, 0:1])
        tch_i = nc.vector.tensor_copy(junk[0:1, 1:2], iota_sb[0:1, 0:1])
        strip_names.append(tch_b.ins.name)
        strip_names.append(tch_i.ins.name)

        # prologue hs DMAs, ordered by first PE use: fp16 tiles lead on the
        # ACT queue; the later fp8 tiles ride the SP queue after the w chunks
        hst16_t = {}
        hc8_t = {}
        for s in range(min(HS_BUFS, ST)):
            hst16_t[s] = hsp.tile([P, H], f16, tag="hst16", name=f"hst16_{s}")
            hc8_t[s] = hsp.tile([P, 2 * HT, P], f8, tag="hc8", name=f"hc8_{s}")
        nc.scalar.dma_start(out=hst16_t[0][:, :H // 2],
                            in_=hsT16[0][:, :H // 2])
        nc.scalar.dma_start(out=hst16_t[0][:, H // 2:],
                            in_=hsT16[0][:, H // 2:])
        nc.scalar.dma_start(out=hst16_t[1], in_=hsT16[1])
        nc.scalar.dma_start(out=hst16_t[2], in_=hsT16[2])
        nc.scalar.dma_start(
            out=hc8_t[0], in_=hsC8[0].rearrange("p (k t) -> p k t", t=P))
        nc.scalar.dma_start(out=hst16_t[3], in_=hsT16[3])
        nc.sync.dma_start(
            out=hc8_t[1], in_=hsC8[1].rearrange("p (k t) -> p k t", t=P))
        nc.sync.dma_start(
            out=hc8_t[2], in_=hsC8[2].rearrange("p (k t) -> p k t", t=P))
        nc.sync.dma_start(
            out=hc8_t[3], in_=hsC8[3].rearrange("p (k t) -> p k t", t=P))

        dummy_ps = dps.tile([1, 1], f32)
        scores_t = {}
        logits_t = {}
        corr_t = {}
        logits_ps_t = {}
        corr_ps_t = {}
        last_act = last_dve = last_mm = None
        next_u = 0

        def emit_back_half(u):
            nonlocal last_act, last_dve, last_mm
            hc8 = hc8_t[u]
            # ---- PE: fp8-DR residual pass for tile u (LAG tiles behind) ----
            if u == 0:
                tpe2 = nc.tensor.matmul(dummy_ps, hc8[0:1, 0, 0:4].bitcast(f32),
                                        hc8[0:1, 0, 0:4].bitcast(f32),
                                        start=True, stop=True)
            elif u >= 2:
                # absorb DVE's release of bank B (merge of tile u-2)
                tpe2 = nc.tensor.matmul(dummy_ps, logits_t[u - 2][0:1, 0:1],
                                        logits_t[u - 2][0:1, 0:1],
                                        start=True, stop=True)
            else:
                tpe2 = None
            corr_ps = cps.tile([P, E], f32, tag="corr", name=f"corr_ps{u}")
            corr_ps_t[u] = corr_ps
            mm = None
            for j in range(HT):
                mm = nc.tensor.matmul(
                    corr_ps,
                    hc8[:, 2 * j:2 * j + 2, :],
                    w8_sb[:, 2 * j:2 * j + 2, :],
                    start=(j == 0), stop=(j == HT - 1),
                    perf_mode=mybir.MatmulPerfMode.DoubleRow)
                if j == 0:
                    _dep(mm, tpe2, sync=False)
            last_mm = mm

            # ---- DVE: merge in place: logits_sb += corr_ps * 2^-13 ----
            # (bank A was already copied to logits_sb right after the fp16
            # pass, off the critical path; one PSUM input satisfies the
            # single-PSUM-read rule and the csc hop disappears)
            logits_sb = logits_t[u]
            # absorb the PE wait (bank B stop) so the merge keeps only its
            # same-engine RAW wait on the bank-A pre-copy
            t_ps8 = nc.vector.tensor_copy(junk_dve[0:1, 2 * u:2 * u + 1],
                                          corr_ps[0:1, 0:1])
            strip_names.append(t_ps8.ins.name)
            mrg = nc.vector.scalar_tensor_tensor(
                out=logits_sb, in0=corr_ps, scalar=RES_ISCALE, in1=logits_sb,
                op0=mybir.AluOpType.mult, op1=mybir.AluOpType.add)
            _dep(mrg, t_ps8, sync=False)

            # ---- ACT: sigmoid ----
            scores = sco.tile([P, E], f32, tag="scores", name=f"scores{u}")
            scores_t[u] = scores
            act = nc.scalar.activation(scores, logits_sb,
                                       mybir.ActivationFunctionType.Sigmoid)
            strip_names.append(act.ins.name)
            last_act = act

            # hs DMAs for tile u+HS_BUFS: ACT issue order after sigmoid(u)
            # guarantees PE is done reading the buffers they recycle.
            sn = u + HS_BUFS
            if sn < ST:
                hc8_t[sn] = hsp.tile([P, 2 * HT, P], f8, tag="hc8",
                                     name=f"hc8_{sn}")
                # SP store-touch waits on PE's release of the recycled buffer
                # (mm8-last of tile u); the DMA then follows in SP issue order
                # and keeps only its own-lane wait.
                stg = nc.sync.store(junk_spg[0:1, u:u + 1], 0)
                _dep(stg, mm, sync=True, reason="hc8 recycle gate")
                d = nc.sync.dma_start(
                    out=hc8_t[sn], in_=hsC8[sn].rearrange("p (k t) -> p k t", t=P))
                _dep(d, stg, sync=False)
                gated_dmas.append(d.ins.name)

            # ---- DVE: DeepSeek group-limited top-k routing ----
            sfc = sco.tile([P, E], f32, tag="sfc", name=f"sfc{u}")
            a1 = nc.vector.tensor_add(sfc, scores, bias_sb)
            if u == 0:
                _dep(a1, tch_b, sync=False)

            grp = sco.tile([P, G, 8], f32, tag="grp", name=f"grp{u}")
            for g in range(G):
                nc.vector.max(out=grp[:, g], in_=sfc[:, g * GS:(g + 1) * GS])
            gsum = sco.tile([P, G], f32, tag="gsum", name=f"gsum{u}")
            nc.vector.tensor_add(gsum, grp[:, :, 0], grp[:, :, 1])
            g8 = sco.tile([P, 8], f32, tag="g8", name=f"g8{u}")
            nc.vector.max(out=g8, in_=gsum)
            # tmp = (group_score >= 4th-largest) * sfc, fused in one STT
            tmp = sco.tile([P, E], f32, tag="tmp", name=f"tmp{u}")
            nc.vector.scalar_tensor_tensor(
                out=tmp.rearrange("p (g s) -> p g s", g=G),
                in0=gsum.rearrange("p (g one) -> p g one", one=1)
                .to_broadcast([P, G, GS]),
                scalar=g8[:, TOPK_GROUP - 1:TOPK_GROUP],
                in1=sfc.rearrange("p (g s) -> p g s", g=G),
                op0=mybir.AluOpType.is_ge,
                op1=mybir.AluOpType.mult)

            t8 = sco.tile([P, TOP_K], f32, tag="t8", name=f"t8{u}")
            ti8 = sco.tile([P, TOP_K], u32, tag="ti8", name=f"ti8{u}")
            nc.vector.max(out=t8, in_=tmp)
            nc.vector.max_index(out=ti8, in_max=t8, in_values=tmp)

            ti8f = sco.tile([P, TOP_K], f32, tag="ti8f", name=f"ti8f{u}")
            nc.vector.tensor_copy(ti8f, ti8)
            wk = sco.tile([P, TOP_K], f32, tag="wk", name=f"wk{u}")
            eqk = sco.tile([P, E], f32, tag="eqk", name=f"eqk{u}")
            for k in range(TOP_K):
                e1 = nc.vector.scalar_tensor_tensor(
                    out=eqk, in0=iota_sb, scalar=ti8f[:, k:k + 1], in1=scores,
                    op0=mybir.AluOpType.is_equal, op1=mybir.AluOpType.mult,
                    accum_out=wk[:, k:k + 1])
                if u == 0 and k == 0:
                    _dep(e1, tch_i, sync=False)

            denom = sco.tile([P, 1], f32, tag="denom", name=f"denom{u}")
            nc.vector.tensor_reduce(denom, wk, axis=mybir.AxisListType.X,
                                    op=mybir.AluOpType.add)
            nc.vector.tensor_scalar_add(denom, denom, EPS)
            rcp = sco.tile([P, 1], f32, tag="rcp", name=f"rcp{u}")
            nc.vector.reciprocal(rcp, denom)
            nc.vector.tensor_scalar(out=wacc[:, u], in0=wk, scalar1=rcp,
                                    scalar2=ROUTED_SCALING,
                                    op0=mybir.AluOpType.mult,
                                    op1=mybir.AluOpType.mult)
            last_dve = nc.vector.tensor_copy(iacc[:, u], ti8)

        for s in range(ST):
            hst = hst16_t[s]
            # ---- PE: fp16 main pass for tile s into bank A ----
            if s == 0:
                # absorb the hst16 DMA lane of tile 0 on PE
                tpe = nc.tensor.matmul(dummy_ps, hst[0:1, 0:1], hst[0:1, 0:1],
                                       start=True, stop=True)
            elif s >= LAG + 2:
                # absorb DVE's release of bank A: bank A of tile s
                # (lps bufs=LAG+2) is released by the merge of tile
                # s-(LAG+2), which ran in iteration s-2.
                tpe = nc.tensor.matmul(dummy_ps,
                                       logits_t[s - (LAG + 2)][0:1, 0:1],
                                       logits_t[s - (LAG + 2)][0:1, 0:1],
                                       start=True, stop=True)
            else:
                tpe = None
            logits_ps = lps.tile([P, E], f32, tag="logits", name=f"logits{s}")
            logits_ps_t[s] = logits_ps
            for h in range(HT):
                mm = nc.tensor.matmul(
                    logits_ps,
                    hst[:, h * P:(h + 1) * P],
                    whi_sb[:, h, :],
                    start=(h == 0), stop=(h == HT - 1))
                if h == 0:
                    _dep(mm, tpe, sync=False)

            # DVE pre-copy of bank A to SBUF, LAG tiles ahead of the merge
            t_cpA = None
            if s >= 3:
                # absorb ACT's release of the logits_sb buffer being recycled
                # (3-buffer rotation: written at iter s, read at iter s+LAG)
                t_cpA = nc.vector.tensor_copy(junk_dve[0:1, 2 * s + 1:2 * s + 2],
                                              scores_t[s - 3][0:1, 0:1])
                strip_names.append(t_cpA.ins.name)
            logits_sb_s = sco.tile([P, E], f32, tag="logits_sb",
                                   name=f"logits_sb{s}", bufs=3)
            logits_t[s] = logits_sb_s
            cpA = nc.vector.tensor_copy(logits_sb_s, logits_ps)
            _dep(cpA, t_cpA, sync=False)
            strip_names.append(cpA.ins.name)

            # Steady state runs LAG tiles behind; the lag collapses over the
            # last iterations so the final fp8 passes + epilogues overlap the
            # last fp16 passes instead of serializing after PE finishes.
            target = min(s, s - LAG + max(0, s - (ST - LAG - 1) + 1))
            while next_u <= target:
                emit_back_half(next_u)
                next_u += 1

            # issue the hst16 DMA that recycles this tile's buffer as soon as
            # the fp16 pass is done with it (ACT touch on the PSUM bank);
            # emitted AFTER the back-half so its 5.5us transfer never blocks
            # csc/sigmoid on the in-order ACT queue
            sn16 = s + HS_BUFS
            if sn16 < ST:
                t_rel = nc.scalar.copy(junk_act[0:1, s:s + 1],
                                       logits_ps[0:1, 0:1])
                strip_names.append(t_rel.ins.name)
                hst16_t[sn16] = hsp.tile([P, H], f16, tag="hst16",
                                         name=f"hst16_{sn16}")
                d = nc.scalar.dma_start(out=hst16_t[sn16], in_=hsT16[sn16])
                _dep(d, t_rel, sync=False)
                gated_dmas.append(d.ins.name)

        with nc.sync.register("sphead") as hreg:
            l1 = nc.sync.load(hreg, wacc.bitcast(i32)[0:1, ST - 1, 0:1])
            l2 = nc.sync.load(hreg, iacc[0:1, ST - 1, 0:1])
        d_ow = nc.sync.dma_start(out=o_w.rearrange("(t p) k -> p t k", p=P), in_=wacc)
        d_oi = nc.sync.dma_start(out=o_i.rearrange("(t p) k -> p t k", p=P), in_=iacc)
        _dep(d_ow, l1, sync=False)
        _dep(d_ow, l2, sync=False)
        _dep(d_oi, l1, sync=False)
        _dep(d_oi, l2, sync=False)
        gated_dmas.append(d_ow.ins.name)
        gated_dmas.append(d_oi.ins.name)
        # Output-DMA completion guard: queue marker DMAs behind the output
        # DMAs on the same SP HWDGE ring (FIFO per ring), then read their
        # SBUF destinations — a real RAW on the marker's completion implies
        # the output DMAs have fully landed.
        # 8 markers: one per DMAHW lane sem, so the final drain's lane waits
        # are all covered by SP's watermark (single-wait rule)
        junk_dma = const.tile([P, 8, 4], i32)
        for m in range(8):
            dm = nc.sync.dma_start(out=junk_dma[:, m],
                                   in_=bias.bitcast(i32)[:, 4 * m:4 * m + 4])
            _dep(dm, d_ow, sync=False)
            _dep(dm, d_oi, sync=False)
        with nc.sync.register("sptail") as rreg:
            for m in range(8):
                nc.sync.load(rreg, junk_dma[0:1, m, 0:1])
        _funnel(nc, [last_act, last_dve, last_mm], junk_sp)

    return nc, strip_names, gated_dmas, pe_only_dmas


def strip_self_waits(nc, only_names=None):
    """Remove sem-ge waits where the waiting engine is the sole updater of
    the semaphore (Tile's same-engine drain guards) for the listed
    instructions, whose real hazard is protected elsewhere."""
    insts = []
    for f in nc.m.functions:
        for b in f.blocks:
            insts.extend(b.instructions)
    from collections import defaultdict
    upd = defaultdict(set)
    for i in insts:
        si = i.sync_info
        if si and si.on_update:
            for u in si.on_update:
                nm = getattr(u, "ant_name", None)
                if nm:
                    upd[nm].add(str(i.engine))
    n = 0
    for i in insts:
        eligible = (only_names is not None and i.name in only_names) or (
            str(i.engine) == "EngineType.SP"
            and type(i).__name__ in ("InstTensorSave", "InstDrain"))
        if not eligible:
            continue
        si = i.sync_info
        if not (si and si.on_wait):
            continue
        keep = [w for w in si.on_wait
                if not (w.wait_mode == "sem-ge-imm"
                        and upd.get(w.ant_name) == {str(i.engine)})]
        if len(keep) != len(si.on_wait):
            si.on_wait = keep
            n += 1
    return n


def strip_non_pe_dma_waits(nc, names):
    """Keep only the PE-sem wait on the listed DMAs (their other waits are
    transitively covered by it)."""
    n = 0
    for f in nc.m.functions:
        for b in f.blocks:
            for i in b.instructions:
                if i.name not in names:
                    continue
                si = i.sync_info
                if not (si and si.on_wait):
                    continue
                keep = [w for w in si.on_wait if w.ant_name.startswith("PE")]
                if len(keep) != len(si.on_wait):
                    si.on_wait = keep
                    n += 1
    return n


def strip_gated_dma_waits(nc, names):
    """For DMAs whose issue is gated by a preceding same-engine instruction
    that already waits on the release dep, keep only the DMA-lane waits."""
    n = 0
    for f in nc.m.functions:
        for b in f.blocks:
            for i in b.instructions:
                if i.name not in names:
                    continue
                si = i.sync_info
                if not (si and si.on_wait):
                    continue
                own = {getattr(u, "ant_name", "") for u in (si.on_update or [])}
                keep = [w for w in si.on_wait if w.ant_name in own]
                if len(keep) != len(si.on_wait):
                    si.on_wait = keep
                    n += 1
    return n


def validate_single_wait(nc, max_waits=1):
    bad = []
    for f in nc.m.functions:
        for b in f.blocks:
            for i in b.instructions:
                si = i.sync_info
                nw = len(si.on_wait) if si and si.on_wait else 0
                if nw > max_waits:
                    dbg = i.debug
                    loc = f"{dbg.filename}:{dbg.lineno}" if dbg else "?"
                    bad.append((i.name, type(i).__name__, str(i.engine), nw, loc,
                                [w.ant_name for w in si.on_wait]))
    return bad


_NC_CACHE = None


def _get_nc():
    global _NC_CACHE
    if _NC_CACHE is None:
        nc, strip_names, gated_dmas, pe_only_dmas = build_program()
        strip_self_waits(nc, only_names=set(strip_names))
        strip_gated_dma_waits(nc, set(gated_dmas))
        strip_non_pe_dma_waits(nc, set(pe_only_dmas))
        bad = validate_single_wait(nc)
        if bad:
            raise RuntimeError(f"{len(bad)} multi-wait instructions: {bad[:5]}")
        _NC_CACHE = nc
    return _NC_CACHE


def _tile_t(x2d):
    """[SL, Hx] -> [ST, P, Hx] with [s, p, kt*128+t] = x[s*128+t, kt*128+p]."""
    ht = x2d.shape[1] // P
    blk = x2d.reshape(ST, P, ht, P)
    return np.ascontiguousarray(blk.transpose(0, 3, 2, 1)).reshape(ST, P, ht * P)


def _prep_inputs(hidden_states, weight, e_score_correction_bias):
    import ml_dtypes
    f8np = ml_dtypes.float8_e4m3

    hs = np.asarray(hidden_states, dtype=np.float32)
    w = np.asarray(weight, dtype=np.float32)
    b = np.asarray(e_score_correction_bias, dtype=np.float32)

    w16 = w.astype(np.float16)
    wlo = w - w16.astype(np.float32)
    w8a = w.astype(f8np)
    w8b = (wlo * np.float32(RES_SCALE)).astype(f8np)

    def wtile(x):
        # [E, H] -> [P, HT, E] flattened to [P, HT*E]
        ht = x.shape[1] // P
        return np.ascontiguousarray(
            x.reshape(E, ht, P).transpose(2, 1, 0)).reshape(P, ht * E)

    whi16_m = wtile(w16)
    w8cat = np.concatenate([wtile(w8a), wtile(w8b)], axis=1)
    bias_b = np.ascontiguousarray(np.broadcast_to(b, (P, E)).astype(np.float32))
    iota = np.ascontiguousarray(
        np.broadcast_to(np.arange(E, dtype=np.float32), (P, E)))

    def core_map(c):
        blk = hs[c * SL:(c + 1) * SL]
        hs16 = blk.astype(np.float16)
        hslo = blk - hs16.astype(np.float32)
        hslo8 = (hslo * np.float32(RES_SCALE)).astype(f8np)
        hs8 = blk.astype(f8np)
        hsT16 = _tile_t(hs16)
        hsC8 = np.ascontiguousarray(
            np.concatenate([_tile_t(hslo8), _tile_t(hs8)], axis=2))
        return {"hsT16": hsT16, "hsC8": hsC8, "whi16": whi16_m,
                "w8": w8cat, "bias": bias_b, "iota": iota}

    from concurrent.futures import ThreadPoolExecutor
    with ThreadPoolExecutor(max_workers=N_CORES) as ex:
        return list(ex.map(core_map, range(N_CORES)))


def run(hidden_states, weight, e_score_correction_bias, trace=False):
    nc = _get_nc()
    in_maps = _prep_inputs(hidden_states, weight, e_score_correction_bias)
    res = run_bass_kernel_spmd(nc, in_maps, list(range(N_CORES)), trace=trace)
    w = np.concatenate([r["o_w"] for r in res.results], axis=0)
    i = np.concatenate([r["o_i"] for r in res.results], axis=0).astype(np.int32)
    return (w, i), res


def kernel(hidden_states, weight, e_score_correction_bias):
    (w, i), _ = run(hidden_states, weight, e_score_correction_bias)
    return w, i

